# revision 16
# baseline (speedup 1.0000x reference)
"""Trainium2 Bass kernel for nn_Attend (decomposable attention).

Computation (reference):
    f_A = relu(relu(A @ W1 + b1) @ W2 + b2)      [b, m, h]
    f_B = relu(relu(B @ W1 + b1) @ W2 + b2)      [b, n, h]
    e = f_A @ f_B^T                               [b, m, n]
    beta  = softmax(e, axis=-1) @ B               [b, m, d]
    alpha = softmax(e, axis=-2)^T @ A             [b, n, d]
    returns (beta, alpha)

Shapes: b=4, m=n=4096, d=128, h=256. Scores e lie in ~[0.5, 8.3] so
exp() needs no max-subtraction: softmax = exp(e)/sum(exp(e)) directly,
and cross-shard softmax combines are plain sums of partials.

Sharding: 8 cores = (batch, m-half). Each core handles 2048 m-rows of
one batch against all 4096 n. beta is fully local (row softmax over n).
alpha needs a sum over m across the 2 cores of a batch: each core emits
unnormalized alpha^T partials + column-sum partials; the host adds the
two partials and divides (exact).

On-core dataflow (all heavy matmuls in float32r: full PE rate at
N=512 moving dim, ~1.6e-4 relative error):
  1. MLP in transposed layout: fB^T [256, 4096], fA^T [256, 2048] as
     2x128-partition f32r tiles. Bias+relu fused on DVE (tensor_scalar
     add+max from PSUM).
  2. alpha phase (natural layout, m-chunk outer, n split in 2 halves):
     E chunk [128m, 512n] on PE -> ACT exp -> P natural (f32r) with
     fused per-m rowsum (accum_out); alpha^T [128d, 2048n] accumulates
     in PSUM over the 16 m-chunks: lhsT=A-natural-block, rhs=P.
  3. beta phase (transposed layout, n-block outer): E^T [128n, 512m]
     on PE -> ACT exp -> P^T (f32r) with fused per-n colsum partials;
     beta^T [128d, 2048m] accumulates in PSUM over the 32 n-blocks:
     lhsT=B-natural-block, rhs=P^T.
  4. finalize: beta^T -> PE transpose per 128-block -> ACT copy scaled
     by 1/rowsum (per-partition scale) -> natural [2048, 128] output.
"""

import os
import sys

import numpy as np

if "/opt/trn_rl_repo" not in sys.path:
    sys.path.insert(0, "/opt/trn_rl_repo")

import concourse.bass as bass  # noqa: E402
import concourse.mybir as mybir  # noqa: E402
import concourse.tile as tile  # noqa: E402
from concourse import bacc  # noqa: E402
from concourse.bass_utils import run_bass_kernel_spmd  # noqa: E402
from concourse.masks import make_identity  # noqa: E402

F32 = mybir.dt.float32
F32R = mybir.dt.float32r
EXP = mybir.ActivationFunctionType.Exp
COPY = mybir.ActivationFunctionType.Copy
RELU = mybir.ActivationFunctionType.Relu

D = 128      # model dim
H = 256      # hidden dim
M = 2048     # rows per core (half of 4096)
N = 4096     # full sequence
MC = M // 128   # m chunks per core (16)
NB = N // 128   # n blocks (32)

_CACHE = {}
LAST_RESULT = None  # BassKernelResults of the most recent run (for test.py)


def _mlp_transposed(nc, pools, xT, fT0, fT1, w1, w2, b1c, b2c, zero, seq):
    """fT = relu(W2^T @ relu(W1^T @ xT + b1) + b2) in transposed layout.

    xT: [128 d, seq] f32r;  fT0/fT1: [128, seq] f32r (h split in 2 blocks).
    h1 bias+relu runs on ACT, h2 bias+relu on DVE (splits the elementwise
    load so neither engine gates the PE).
    """
    mlp_ps, h1_pool = pools
    h1 = [h1_pool.tile([128, seq], F32R, tag=f"h1_{i}", name=f"h1_{i}")
          for i in range(2)]
    nchunks = seq // 512
    for s in range(nchunks):
        sl = bass.ts(s, 512)
        for i in range(2):
            ps = mlp_ps.tile([128, 512], F32, tag="mlp")
            nc.tensor.matmul(ps, w1[:, bass.ts(i, 128)], xT[:, sl],
                             start=True, stop=True)
            nc.scalar.activation(h1[i][:, sl], ps, RELU,
                                 bias=b1c[:, i:i + 1])
        for i, fT in enumerate((fT0, fT1)):
            if s == 0:
                continue
            ps = mlp_ps.tile([128, 512], F32, tag="mlp")
            sl_prev = bass.ts(s - 1, 512)
            for kh in range(2):
                nc.tensor.matmul(ps, w2[:, bass.ts(kh * 2 + i, 128)],
                                 h1[kh][:, sl_prev],
                                 start=(kh == 0), stop=(kh == 1))
            nc.vector.tensor_scalar(
                out=fT[:, sl_prev], in0=ps,
                scalar1=b2c[:, i:i + 1], scalar2=zero,
                op0=mybir.AluOpType.add, op1=mybir.AluOpType.max)
    sl = bass.ts(nchunks - 1, 512)
    for i, fT in enumerate((fT0, fT1)):
        ps = mlp_ps.tile([128, 512], F32, tag="mlp")
        for kh in range(2):
            nc.tensor.matmul(ps, w2[:, bass.ts(kh * 2 + i, 128)],
                             h1[kh][:, sl],
                             start=(kh == 0), stop=(kh == 1))
        nc.vector.tensor_scalar(
            out=fT[:, sl], in0=ps,
            scalar1=b2c[:, i:i + 1], scalar2=zero,
            op0=mybir.AluOpType.add, op1=mybir.AluOpType.max)


def _build():
    """Build + compile the per-core Bass program (same NEFF on all 8 cores)."""
    nc = bacc.Bacc(None, target_bir_lowering=False)

    # inputs (f32r tensors take plain fp32 host arrays)
    atb = nc.declare_dram_parameter("atb", [128, M], F32R, isOutput=False)
    btb = nc.declare_dram_parameter("btb", [128, N], F32R, isOutput=False)
    anr = nc.declare_dram_parameter("anr", [128, M], F32R, isOutput=False)
    bnr = nc.declare_dram_parameter("bnr", [128, N], F32R, isOutput=False)
    w1 = nc.declare_dram_parameter("w1", [128, H], F32R, isOutput=False)
    w2 = nc.declare_dram_parameter("w2", [128, 2 * H], F32R, isOutput=False)
    b1 = nc.declare_dram_parameter("b1", [128, 2], F32, isOutput=False)
    b2 = nc.declare_dram_parameter("b2", [128, 2], F32, isOutput=False)
    # outputs
    beta_d = nc.declare_dram_parameter("beta", [M, D], F32, isOutput=True)
    alphat_d = nc.declare_dram_parameter("alphat", [128, N], F32, isOutput=True)
    cols_d = nc.declare_dram_parameter("cols", [128, 2 * NB], F32, isOutput=True)

    with tile.TileContext(nc) as tc, \
         tc.tile_pool(name="const", bufs=1) as const:
        # persistent SBUF tensors
        atb_sb = const.tile([128, M], F32R, tag="atb")
        btb_sb = const.tile([128, N], F32R, tag="btb")
        anr_sb = const.tile([128, M], F32R, tag="anr")
        bnr_sb = const.tile([128, N], F32R, tag="bnr")
        w1_sb = const.tile([128, H], F32R, tag="w1")
        w2_sb = const.tile([128, 2 * H], F32R, tag="w2")
        b1_sb = const.tile([128, 2], F32, tag="b1")
        b2_sb = const.tile([128, 2], F32, tag="b2")

        zero = const.tile([128, 1], F32, tag="zero")
        nc.vector.memset(zero, 0.0)
        # trigger the exp table-set load on ACT immediately (overlaps with
        # the input DMAs instead of stalling the first real exp)
        dummy = const.tile([128, 1], F32, tag="dummy")
        nc.scalar.activation(dummy, zero, EXP)

        # DMA issue costs ~650ns/instr on the SP sequencer and transfers
        # drain through a mostly-serial DMA pipe, so: small weight tensors
        # and the first MLP operand chunk first, big tensors in few chunks,
        # ordered by first use.
        nc.sync.dma_start(w1_sb, w1[:])
        nc.sync.dma_start(btb_sb[:, 0:512], btb[:, 0:512])
        nc.sync.dma_start(b1_sb, b1[:])
        nc.sync.dma_start(w2_sb, w2[:])
        nc.sync.dma_start(b2_sb, b2[:])
        nc.sync.dma_start(btb_sb[:, 512:1536], btb[:, 512:1536])
        nc.sync.dma_start(btb_sb[:, 1536:2560], btb[:, 1536:2560])
        nc.sync.dma_start(btb_sb[:, 2560:4096], btb[:, 2560:4096])
        nc.sync.dma_start(atb_sb, atb[:])
        nc.sync.dma_start(anr_sb, anr[:])
        nc.sync.dma_start(bnr_sb, bnr[:])

        fbt = [const.tile([128, N], F32R, tag=f"fbt{k}", name=f"fbt{k}")
               for k in range(2)]
        fat = [const.tile([128, M], F32R, tag=f"fat{k}", name=f"fat{k}")
               for k in range(2)]
        rows_sb = const.tile([128, 4 * MC], F32, tag="rows")
        cols_sb = const.tile([128, 2 * NB], F32, tag="cols")

        # ---- phase 1: MLPs ----
        with tc.tile_pool(name="mlp_ps", bufs=4, space="PSUM") as mlp_ps, \
             tc.tile_pool(name="h1", bufs=1) as h1_pool:
            pools = (mlp_ps, h1_pool)
            _mlp_transposed(nc, pools, btb_sb, fbt[0], fbt[1],
                            w1_sb, w2_sb, b1_sb, b2_sb, zero, N)
            _mlp_transposed(nc, pools, atb_sb, fat[0], fat[1],
                            w1_sb, w2_sb, b1_sb, b2_sb, zero, M)

        # ---- phases 2+3 share one PSUM/SBUF pool set: the beta phase's
        # tiles reuse the alpha phase's slots as they free, with no
        # pool-close barrier at the transition ----
        rs_tmp = const.tile([128, 2 * MC], F32, tag="rst")
        rs = const.tile([128, MC], F32, tag="rs")
        inv_rs = const.tile([128, MC], F32, tag="irs")
        ident = const.tile([128, 128], F32, tag="id")
        make_identity(nc, ident)
        with tc.tile_pool(name="acc_ps", bufs=1, space="PSUM") as acc_pool, \
             tc.tile_pool(name="et_ps", bufs=2, space="PSUM") as et_pool, \
             tc.tile_pool(name="pp", bufs=3) as ppool, \
             tc.tile_pool(name="stage", bufs=4) as stage, \
             tc.tile_pool(name="fin", bufs=1) as fin:
            # -- alpha + rowsums --
            for nh in range(2):
                # 4 single-bank accumulators so finished slices copy out and
                # free PSUM without waiting for the whole [128, 2048] strip
                alpha_ps = [acc_pool.tile([128, 512], F32, tag=f"acc{q}",
                                          name=f"acc{q}") for q in range(4)]
                for c in range(MC):
                    p_c = ppool.tile([128, 2048], F32R, tag="P")
                    for sub in range(2):
                        et = et_pool.tile([128, 1024], F32, tag="et")
                        for h2 in range(2):
                            ncol = nh * 2048 + sub * 1024 + h2 * 512
                            for k in range(2):
                                nc.tensor.matmul(
                                    et[:, bass.ts(h2, 512)],
                                    fat[k][:, bass.ts(c, 128)],
                                    fbt[k][:, ncol:ncol + 512],
                                    start=(k == 0), stop=(k == 1))
                        nc.scalar.activation(
                            p_c[:, bass.ts(sub, 1024)], et, EXP,
                            accum_out=rows_sb[:, nh * 2 * MC + c * 2 + sub
                                              : nh * 2 * MC + c * 2 + sub + 1])
                    for q in range(4):
                        nc.tensor.matmul(
                            alpha_ps[q],
                            anr_sb[:, bass.ts(c, 128)],
                            p_c[:, bass.ts(q, 512)],
                            start=(c == 0), stop=(c == MC - 1))
                for q in range(4):
                    a_sb = stage.tile([128, 512], F32, tag="as")
                    nc.vector.tensor_copy(a_sb, alpha_ps[q])
                    nc.sync.dma_start(
                        alphat_d[:, nh * 2048 + q * 512:nh * 2048 + (q + 1) * 512],
                        a_sb)

            # rowsum -> 1/rowsum as soon as the alpha-phase exps finish
            # (off the critical path of the beta phase)
            nc.vector.tensor_add(rs_tmp, rows_sb[:, 0:2 * MC],
                                 rows_sb[:, 2 * MC:4 * MC])
            rs_pairs = rs_tmp.rearrange("p (c s) -> p c s", s=2)
            nc.vector.tensor_add(rs, rs_pairs[:, :, 0], rs_pairs[:, :, 1])
            nc.vector.reciprocal(inv_rs, rs)

            # -- beta + colsums (accumulators reuse the acc0-3 slots) --
            beta_ps = [acc_pool.tile([128, 512], F32, tag=f"acc{q}",
                                     name=f"bacc{q}") for q in range(4)]
            betat_sb = fin.tile([128, M], F32, tag="bt")
            for j in range(NB):
                pt_j = ppool.tile([128, M], F32R, tag="P")
                for sub in range(2):
                    et = et_pool.tile([128, 1024], F32, tag="et")
                    for h2 in range(2):
                        mcol = sub * 1024 + h2 * 512
                        for k in range(2):
                            nc.tensor.matmul(
                                et[:, bass.ts(h2, 512)],
                                fbt[k][:, bass.ts(j, 128)],
                                fat[k][:, mcol:mcol + 512],
                                start=(k == 0), stop=(k == 1))
                    nc.scalar.activation(
                        pt_j[:, bass.ts(sub, 1024)], et, EXP,
                        accum_out=cols_sb[:, j * 2 + sub:j * 2 + sub + 1])
                for q in range(4):
                    nc.tensor.matmul(
                        beta_ps[q],
                        bnr_sb[:, bass.ts(j, 128)],
                        pt_j[:, bass.ts(q, 512)],
                        start=(j == 0), stop=(j == NB - 1))
            nc.sync.dma_start(cols_d[:], cols_sb)
            for q in range(4):
                nc.vector.tensor_copy(betat_sb[:, bass.ts(q, 512)],
                                      beta_ps[q])

            # -- finalize beta: transpose blocks + scale by 1/rowsum;
            # per-block DMA so stores overlap the transposes --
            beta_out = fin.tile([128, MC, 128], F32, tag="bout")
            beta_nat = beta_d[:].rearrange("(c p) d -> p c d", p=128)
            for c in range(MC):
                tr = et_pool.tile([128, 128], F32, tag="et", name=f"tr{c}")
                nc.tensor.transpose(tr, betat_sb[:, bass.ts(c, 128)], ident)
                nc.vector.tensor_scalar_mul(beta_out[:, c, :], tr,
                                            inv_rs[:, c:c + 1])
                nc.sync.dma_start(beta_nat[:, c, :], beta_out[:, c, :])

    nc.compile()
    return nc


def _get_nc():
    if "nc" not in _CACHE:
        _CACHE["nc"] = _build()
    return _CACHE["nc"]


def _get_runner():
    """Jitted 8-core shard_map executor built once (mirrors
    bass2jax.run_bass_via_pjrt, but cacheable across calls)."""
    if "runner" in _CACHE:
        return _CACHE["runner"]
    import jax
    from jax.sharding import Mesh, PartitionSpec
    from jax.experimental.shard_map import shard_map
    import concourse.mybir as mb
    from concourse.bass2jax import (
        _bass_exec_p, install_neuronx_cc_hook, partition_id_tensor)

    nc = _get_nc()
    install_neuronx_cc_hook()

    in_names, out_names, out_avals = [], [], []
    partition_name = (nc.partition_id_tensor.name
                      if nc.partition_id_tensor else None)
    for alloc in nc.m.functions[0].allocations:
        if not isinstance(alloc, mb.MemoryLocationSet):
            continue
        name = alloc.memorylocations[0].name
        if alloc.kind == "ExternalInput":
            if name != partition_name:
                in_names.append(name)
        elif alloc.kind == "ExternalOutput":
            out_names.append(name)
            out_avals.append(jax.core.ShapedArray(
                tuple(alloc.tensor_shape), mb.dt.np(alloc.dtype)))
    n_params = len(in_names)
    zero_outs = [np.zeros((8 * a.shape[0], *a.shape[1:]), a.dtype)
                 for a in out_avals]
    all_in_names = in_names + out_names
    if partition_name is not None:
        all_in_names = all_in_names + [partition_name]

    def _body(*args):
        operands = list(args)
        if partition_name is not None:
            operands.append(partition_id_tensor())
        return tuple(_bass_exec_p.bind(
            *operands,
            out_avals=tuple(out_avals),
            in_names=tuple(all_in_names),
            out_names=tuple(out_names),
            lowering_input_output_aliases=(),
            sim_require_finite=True,
            sim_require_nnan=True,
            nc=nc,
        ))

    devices = jax.devices()[:8]
    mesh = Mesh(np.asarray(devices), ("core",))
    nin = n_params + len(out_names)
    sharded = jax.jit(shard_map(
        _body, mesh=mesh,
        in_specs=(PartitionSpec("core"),) * nin,
        out_specs=(PartitionSpec("core"),) * len(out_names),
        check_rep=False))
    zeros_dev = [jax.device_put(z) for z in zero_outs]
    _CACHE["runner"] = (sharded, in_names, out_names, out_avals, zeros_dev)
    return _CACHE["runner"]


def run_cores(in_maps):
    """Run the 8-core program; returns list of per-core output dicts."""
    import jax
    sharded, in_names, out_names, out_avals, zeros_dev = _get_runner()
    concat_in = [np.concatenate([m[name] for m in in_maps], axis=0)
                 for name in in_names]
    out_arrs = sharded(*concat_in, *zeros_dev)
    out_arrs = [np.asarray(o) for o in out_arrs]
    return [
        {name: out_arrs[i].reshape(8, *out_avals[i].shape)[c]
         for i, name in enumerate(out_names)}
        for c in range(8)
    ]


def build_in_maps(A, B, W1, b1, W2, b2):
    A = np.ascontiguousarray(np.asarray(A, dtype=np.float32))
    B = np.ascontiguousarray(np.asarray(B, dtype=np.float32))
    W1 = np.asarray(W1, dtype=np.float32)
    b1 = np.asarray(b1, dtype=np.float32)
    W2 = np.asarray(W2, dtype=np.float32)
    b2 = np.asarray(b2, dtype=np.float32)
    nbatch, seq, d = A.shape
    assert (nbatch, seq, d) == (4, N, D), (nbatch, seq, d)

    w1r = np.ascontiguousarray(W1)                                # [128, 256]
    w2r = np.ascontiguousarray(
        W2.reshape(2, 128, 2, 128).transpose(1, 0, 2, 3).reshape(128, 512))
    b1c = np.ascontiguousarray(b1.reshape(2, 128).T)              # [128, 2]
    b2c = np.ascontiguousarray(b2.reshape(2, 128).T)

    in_maps = []
    for core in range(8):
        b_i, half = divmod(core, 2)
        Ah = A[b_i, half * M:(half + 1) * M]                      # [2048, 128]
        Bf = B[b_i]                                               # [4096, 128]
        in_maps.append({
            "atb": np.ascontiguousarray(Ah.T),
            "btb": np.ascontiguousarray(Bf.T),
            "anr": np.ascontiguousarray(
                Ah.reshape(MC, 128, 128).transpose(1, 0, 2).reshape(128, M)),
            "bnr": np.ascontiguousarray(
                Bf.reshape(NB, 128, 128).transpose(1, 0, 2).reshape(128, N)),
            "w1": w1r, "w2": w2r, "b1": b1c, "b2": b2c,
        })
    return in_maps


def kernel(A, B, W1, b1, W2, b2):
    in_maps = build_in_maps(A, B, W1, b1, W2, b2)
    results = run_cores(in_maps)

    beta = np.empty((4, N, D), dtype=np.float32)
    alpha = np.empty((4, N, D), dtype=np.float32)
    for b_i in range(4):
        r0 = results[2 * b_i]
        r1 = results[2 * b_i + 1]
        beta[b_i, :M] = r0["beta"]
        beta[b_i, M:] = r1["beta"]
        num = r0["alphat"] + r1["alphat"]                          # [128, 4096]
        colp = r0["cols"] + r1["cols"]                             # [128, 64]
        csum = colp.reshape(128, NB, 2).sum(axis=2)                # [128, 32]
        # csum[p, j] corresponds to n = j*128 + p
        alpha[b_i] = (num / csum.T.reshape(1, N)).T.reshape(N, D)
    return beta, alpha


if __name__ == "__main__":
    rng = np.random.default_rng(0)
    A = rng.standard_normal((4, N, D)).astype(np.float32)
    B = rng.standard_normal((4, N, D)).astype(np.float32)
    s1, s2 = 1.0 / np.sqrt(D), 1.0 / np.sqrt(H)
    W1 = rng.uniform(-s1, s1, (D, H)).astype(np.float32)
    b1 = rng.uniform(-s1, s1, H).astype(np.float32)
    W2 = rng.uniform(-s2, s2, (H, H)).astype(np.float32)
    b2 = rng.uniform(-s2, s2, H).astype(np.float32)
    beta, alpha = kernel(A=A, B=B, W1=W1, b1=b1, W2=W2, b2=b2)
    print("beta", beta.shape, "alpha", alpha.shape)


# revision 17
# speedup vs baseline: 88.6235x; 88.6235x over previous
"""Trainium2 Bass kernel for nn_Attend (decomposable attention).

Computation (reference):
    f_A = relu(relu(A @ W1 + b1) @ W2 + b2)      [b, m, h]
    f_B = relu(relu(B @ W1 + b1) @ W2 + b2)      [b, n, h]
    e = f_A @ f_B^T                               [b, m, n]
    beta  = softmax(e, axis=-1) @ B               [b, m, d]
    alpha = softmax(e, axis=-2)^T @ A             [b, n, d]
    returns (beta, alpha)

Shapes: b=4, m=n=4096, d=128, h=256. Scores e lie in ~[0.5, 8.3] so
exp() needs no max-subtraction: softmax = exp(e)/sum(exp(e)) directly,
and cross-shard softmax combines are plain sums of partials.

Sharding: 8 cores = (batch, m-half). Each core handles 2048 m-rows of
one batch against all 4096 n. beta is fully local (row softmax over n).
alpha needs a sum over m across the 2 cores of a batch: each core emits
unnormalized alpha^T partials + column-sum partials; the host adds the
two partials and divides (exact).

On-core dataflow (all heavy matmuls in float32r: full PE rate at
N=512 moving dim, ~1.6e-4 relative error):
  1. MLP in transposed layout: fB^T [256, 4096], fA^T [256, 2048] as
     2x128-partition f32r tiles. Bias+relu fused on DVE (tensor_scalar
     add+max from PSUM).
  2. alpha phase (natural layout, m-chunk outer, n split in 2 halves):
     E chunk [128m, 512n] on PE -> ACT exp -> P natural (f32r) with
     fused per-m rowsum (accum_out); alpha^T [128d, 2048n] accumulates
     in PSUM over the 16 m-chunks: lhsT=A-natural-block, rhs=P.
  3. beta phase (transposed layout, n-block outer): E^T [128n, 512m]
     on PE -> ACT exp -> P^T (f32r) with fused per-n colsum partials;
     beta^T [128d, 2048m] accumulates in PSUM over the 32 n-blocks:
     lhsT=B-natural-block, rhs=P^T.
  4. finalize: beta^T -> PE transpose per 128-block -> ACT copy scaled
     by 1/rowsum (per-partition scale) -> natural [2048, 128] output.
"""

import os
import sys

import numpy as np

if "/opt/trn_rl_repo" not in sys.path:
    sys.path.insert(0, "/opt/trn_rl_repo")

import concourse.bass as bass  # noqa: E402
import concourse.mybir as mybir  # noqa: E402
import concourse.tile as tile  # noqa: E402
from concourse import bacc  # noqa: E402
from concourse.bass_utils import run_bass_kernel_spmd  # noqa: E402
from concourse.masks import make_identity  # noqa: E402

F32 = mybir.dt.float32
F32R = mybir.dt.float32r
EXP = mybir.ActivationFunctionType.Exp
COPY = mybir.ActivationFunctionType.Copy
RELU = mybir.ActivationFunctionType.Relu

D = 128      # model dim
H = 256      # hidden dim
M = 2048     # rows per core (half of 4096)
N = 4096     # full sequence
MC = M // 128   # m chunks per core (16)
NB = N // 128   # n blocks (32)

_CACHE = {}
LAST_RESULT = None  # BassKernelResults of the most recent run (for test.py)


def _mlp_transposed(nc, pools, xT, fT0, fT1, w1, w2, b1c, b2c, zero, seq):
    """fT = relu(W2^T @ relu(W1^T @ xT + b1) + b2) in transposed layout.

    xT: [128 d, seq] f32r;  fT0/fT1: [128, seq] f32r (h split in 2 blocks).
    h1 bias+relu runs on ACT, h2 bias+relu on DVE (splits the elementwise
    load so neither engine gates the PE).
    """
    mlp_ps, h1_pool = pools
    h1 = [h1_pool.tile([128, seq], F32R, tag=f"h1_{i}", name=f"h1_{i}")
          for i in range(2)]
    nchunks = seq // 512
    for s in range(nchunks):
        sl = bass.ts(s, 512)
        for i in range(2):
            ps = mlp_ps.tile([128, 512], F32, tag="mlp")
            nc.tensor.matmul(ps, w1[:, bass.ts(i, 128)], xT[:, sl],
                             start=True, stop=True)
            nc.scalar.activation(h1[i][:, sl], ps, RELU,
                                 bias=b1c[:, i:i + 1])
        for i, fT in enumerate((fT0, fT1)):
            if s == 0:
                continue
            ps = mlp_ps.tile([128, 512], F32, tag="mlp")
            sl_prev = bass.ts(s - 1, 512)
            for kh in range(2):
                nc.tensor.matmul(ps, w2[:, bass.ts(kh * 2 + i, 128)],
                                 h1[kh][:, sl_prev],
                                 start=(kh == 0), stop=(kh == 1))
            nc.vector.tensor_scalar(
                out=fT[:, sl_prev], in0=ps,
                scalar1=b2c[:, i:i + 1], scalar2=zero,
                op0=mybir.AluOpType.add, op1=mybir.AluOpType.max)
    sl = bass.ts(nchunks - 1, 512)
    for i, fT in enumerate((fT0, fT1)):
        ps = mlp_ps.tile([128, 512], F32, tag="mlp")
        for kh in range(2):
            nc.tensor.matmul(ps, w2[:, bass.ts(kh * 2 + i, 128)],
                             h1[kh][:, sl],
                             start=(kh == 0), stop=(kh == 1))
        nc.vector.tensor_scalar(
            out=fT[:, sl], in0=ps,
            scalar1=b2c[:, i:i + 1], scalar2=zero,
            op0=mybir.AluOpType.add, op1=mybir.AluOpType.max)


def _build():
    """Build + compile the per-core Bass program (same NEFF on all 8 cores)."""
    nc = bacc.Bacc(None, target_bir_lowering=False)

    # inputs (f32r tensors take plain fp32 host arrays)
    atb = nc.declare_dram_parameter("atb", [128, M], F32R, isOutput=False)
    btb = nc.declare_dram_parameter("btb", [128, N], F32R, isOutput=False)
    anr = nc.declare_dram_parameter("anr", [128, M], F32R, isOutput=False)
    bnr = nc.declare_dram_parameter("bnr", [128, N], F32R, isOutput=False)
    w1 = nc.declare_dram_parameter("w1", [128, H], F32R, isOutput=False)
    w2 = nc.declare_dram_parameter("w2", [128, 2 * H], F32R, isOutput=False)
    b1 = nc.declare_dram_parameter("b1", [128, 2], F32, isOutput=False)
    b2 = nc.declare_dram_parameter("b2", [128, 2], F32, isOutput=False)
    # outputs
    beta_d = nc.declare_dram_parameter("beta", [M, D], F32, isOutput=True)
    alphat_d = nc.declare_dram_parameter("alphat", [128, N], F32, isOutput=True)
    cols_d = nc.declare_dram_parameter("cols", [128, 2 * NB], F32, isOutput=True)

    with tile.TileContext(nc) as tc, \
         tc.tile_pool(name="const", bufs=1) as const:
        # persistent SBUF tensors
        atb_sb = const.tile([128, M], F32R, tag="atb")
        btb_sb = const.tile([128, N], F32R, tag="btb")
        anr_sb = const.tile([128, M], F32R, tag="anr")
        bnr_sb = const.tile([128, N], F32R, tag="bnr")
        w1_sb = const.tile([128, H], F32R, tag="w1")
        w2_sb = const.tile([128, 2 * H], F32R, tag="w2")
        b1_sb = const.tile([128, 2], F32, tag="b1")
        b2_sb = const.tile([128, 2], F32, tag="b2")

        zero = const.tile([128, 1], F32, tag="zero")
        nc.vector.memset(zero, 0.0)
        # trigger the exp table-set load on ACT immediately (overlaps with
        # the input DMAs instead of stalling the first real exp)
        dummy = const.tile([128, 1], F32, tag="dummy")
        nc.scalar.activation(dummy, zero, EXP)

        # DMA issue costs ~650ns/instr on the SP sequencer and transfers
        # drain through a mostly-serial DMA pipe, so: small weight tensors
        # and the first MLP operand chunk first, big tensors in few chunks,
        # ordered by first use.
        nc.sync.dma_start(w1_sb, w1[:])
        nc.sync.dma_start(btb_sb[:, 0:512], btb[:, 0:512])
        nc.sync.dma_start(b1_sb, b1[:])
        nc.sync.dma_start(w2_sb, w2[:])
        nc.sync.dma_start(b2_sb, b2[:])
        nc.sync.dma_start(btb_sb[:, 512:1024], btb[:, 512:1024])
        nc.sync.dma_start(btb_sb[:, 1024:2048], btb[:, 1024:2048])
        nc.sync.dma_start(btb_sb[:, 2048:4096], btb[:, 2048:4096])
        nc.sync.dma_start(atb_sb, atb[:])
        nc.sync.dma_start(anr_sb, anr[:])
        nc.sync.dma_start(bnr_sb, bnr[:])

        fbt = [const.tile([128, N], F32R, tag=f"fbt{k}", name=f"fbt{k}")
               for k in range(2)]
        fat = [const.tile([128, M], F32R, tag=f"fat{k}", name=f"fat{k}")
               for k in range(2)]
        rows_sb = const.tile([128, 4 * MC], F32, tag="rows")
        cols_sb = const.tile([128, 2 * NB], F32, tag="cols")

        # ---- phase 1: MLPs ----
        with tc.tile_pool(name="mlp_ps", bufs=4, space="PSUM") as mlp_ps, \
             tc.tile_pool(name="h1", bufs=1) as h1_pool:
            pools = (mlp_ps, h1_pool)
            _mlp_transposed(nc, pools, btb_sb, fbt[0], fbt[1],
                            w1_sb, w2_sb, b1_sb, b2_sb, zero, N)
            _mlp_transposed(nc, pools, atb_sb, fat[0], fat[1],
                            w1_sb, w2_sb, b1_sb, b2_sb, zero, M)

        # ---- phases 2+3 share one PSUM/SBUF pool set: the beta phase's
        # tiles reuse the alpha phase's slots as they free, with no
        # pool-close barrier at the transition ----
        rs_tmp = const.tile([128, 2 * MC], F32, tag="rst")
        rs = const.tile([128, MC], F32, tag="rs")
        inv_rs = const.tile([128, MC], F32, tag="irs")
        ident = const.tile([128, 128], F32, tag="id")
        make_identity(nc, ident)
        with tc.tile_pool(name="acc_ps", bufs=1, space="PSUM") as acc_pool, \
             tc.tile_pool(name="et_ps", bufs=2, space="PSUM") as et_pool, \
             tc.tile_pool(name="pp", bufs=3) as ppool, \
             tc.tile_pool(name="stage", bufs=4) as stage, \
             tc.tile_pool(name="fin", bufs=1) as fin:
            # -- alpha + rowsums --
            for nh in range(2):
                # 4 single-bank accumulators so finished slices copy out and
                # free PSUM without waiting for the whole [128, 2048] strip
                alpha_ps = [acc_pool.tile([128, 512], F32, tag=f"acc{q}",
                                          name=f"acc{q}") for q in range(4)]
                for c in range(MC):
                    p_c = ppool.tile([128, 2048], F32R, tag="P")
                    for sub in range(2):
                        et = et_pool.tile([128, 1024], F32, tag="et")
                        for h2 in range(2):
                            ncol = nh * 2048 + sub * 1024 + h2 * 512
                            for k in range(2):
                                nc.tensor.matmul(
                                    et[:, bass.ts(h2, 512)],
                                    fat[k][:, bass.ts(c, 128)],
                                    fbt[k][:, ncol:ncol + 512],
                                    start=(k == 0), stop=(k == 1))
                        nc.scalar.activation(
                            p_c[:, bass.ts(sub, 1024)], et, EXP,
                            accum_out=rows_sb[:, nh * 2 * MC + c * 2 + sub
                                              : nh * 2 * MC + c * 2 + sub + 1])
                    for q in range(4):
                        nc.tensor.matmul(
                            alpha_ps[q],
                            anr_sb[:, bass.ts(c, 128)],
                            p_c[:, bass.ts(q, 512)],
                            start=(c == 0), stop=(c == MC - 1))
                for q in range(4):
                    a_sb = stage.tile([128, 512], F32, tag="as")
                    nc.vector.tensor_copy(a_sb, alpha_ps[q])
                    nc.sync.dma_start(
                        alphat_d[:, nh * 2048 + q * 512:nh * 2048 + (q + 1) * 512],
                        a_sb)

            # rowsum -> 1/rowsum as soon as the alpha-phase exps finish
            # (off the critical path of the beta phase)
            nc.vector.tensor_add(rs_tmp, rows_sb[:, 0:2 * MC],
                                 rows_sb[:, 2 * MC:4 * MC])
            rs_pairs = rs_tmp.rearrange("p (c s) -> p c s", s=2)
            nc.vector.tensor_add(rs, rs_pairs[:, :, 0], rs_pairs[:, :, 1])
            nc.vector.reciprocal(inv_rs, rs)

            # -- beta + colsums (accumulators reuse the acc0-3 slots) --
            beta_ps = [acc_pool.tile([128, 512], F32, tag=f"acc{q}",
                                     name=f"bacc{q}") for q in range(4)]
            betat_sb = fin.tile([128, M], F32, tag="bt")
            for j in range(NB):
                pt_j = ppool.tile([128, M], F32R, tag="P")
                for sub in range(2):
                    et = et_pool.tile([128, 1024], F32, tag="et")
                    for h2 in range(2):
                        mcol = sub * 1024 + h2 * 512
                        for k in range(2):
                            nc.tensor.matmul(
                                et[:, bass.ts(h2, 512)],
                                fbt[k][:, bass.ts(j, 128)],
                                fat[k][:, mcol:mcol + 512],
                                start=(k == 0), stop=(k == 1))
                    nc.scalar.activation(
                        pt_j[:, bass.ts(sub, 1024)], et, EXP,
                        accum_out=cols_sb[:, j * 2 + sub:j * 2 + sub + 1])
                for q in range(4):
                    nc.tensor.matmul(
                        beta_ps[q],
                        bnr_sb[:, bass.ts(j, 128)],
                        pt_j[:, bass.ts(q, 512)],
                        start=(j == 0), stop=(j == NB - 1))
            nc.sync.dma_start(cols_d[:], cols_sb)
            for q in range(4):
                nc.vector.tensor_copy(betat_sb[:, bass.ts(q, 512)],
                                      beta_ps[q])

            # -- finalize beta: transpose blocks + scale by 1/rowsum;
            # per-block DMA so stores overlap the transposes --
            beta_out = fin.tile([128, MC, 128], F32, tag="bout")
            beta_nat = beta_d[:].rearrange("(c p) d -> p c d", p=128)
            for c in range(MC):
                tr = et_pool.tile([128, 128], F32, tag="et", name=f"tr{c}")
                nc.tensor.transpose(tr, betat_sb[:, bass.ts(c, 128)], ident)
                nc.vector.tensor_scalar_mul(beta_out[:, c, :], tr,
                                            inv_rs[:, c:c + 1])
                nc.sync.dma_start(beta_nat[:, c, :], beta_out[:, c, :])

    nc.compile()
    return nc


def _get_nc():
    if "nc" not in _CACHE:
        _CACHE["nc"] = _build()
    return _CACHE["nc"]


def _get_runner():
    """Jitted 8-core shard_map executor built once (mirrors
    bass2jax.run_bass_via_pjrt, but cacheable across calls)."""
    if "runner" in _CACHE:
        return _CACHE["runner"]
    import jax
    from jax.sharding import Mesh, PartitionSpec
    from jax.experimental.shard_map import shard_map
    import concourse.mybir as mb
    from concourse.bass2jax import (
        _bass_exec_p, install_neuronx_cc_hook, partition_id_tensor)

    nc = _get_nc()
    install_neuronx_cc_hook()

    in_names, out_names, out_avals = [], [], []
    partition_name = (nc.partition_id_tensor.name
                      if nc.partition_id_tensor else None)
    for alloc in nc.m.functions[0].allocations:
        if not isinstance(alloc, mb.MemoryLocationSet):
            continue
        name = alloc.memorylocations[0].name
        if alloc.kind == "ExternalInput":
            if name != partition_name:
                in_names.append(name)
        elif alloc.kind == "ExternalOutput":
            out_names.append(name)
            out_avals.append(jax.core.ShapedArray(
                tuple(alloc.tensor_shape), mb.dt.np(alloc.dtype)))
    n_params = len(in_names)
    zero_outs = [np.zeros((8 * a.shape[0], *a.shape[1:]), a.dtype)
                 for a in out_avals]
    all_in_names = in_names + out_names
    if partition_name is not None:
        all_in_names = all_in_names + [partition_name]

    def _body(*args):
        operands = list(args)
        if partition_name is not None:
            operands.append(partition_id_tensor())
        return tuple(_bass_exec_p.bind(
            *operands,
            out_avals=tuple(out_avals),
            in_names=tuple(all_in_names),
            out_names=tuple(out_names),
            lowering_input_output_aliases=(),
            sim_require_finite=True,
            sim_require_nnan=True,
            nc=nc,
        ))

    devices = jax.devices()[:8]
    mesh = Mesh(np.asarray(devices), ("core",))
    nin = n_params + len(out_names)
    sharded = jax.jit(shard_map(
        _body, mesh=mesh,
        in_specs=(PartitionSpec("core"),) * nin,
        out_specs=(PartitionSpec("core"),) * len(out_names),
        check_rep=False))
    zeros_dev = [jax.device_put(z) for z in zero_outs]
    _CACHE["runner"] = (sharded, in_names, out_names, out_avals, zeros_dev)
    return _CACHE["runner"]


def run_cores(in_maps):
    """Run the 8-core program; returns list of per-core output dicts."""
    import jax
    sharded, in_names, out_names, out_avals, zeros_dev = _get_runner()
    concat_in = [np.concatenate([m[name] for m in in_maps], axis=0)
                 for name in in_names]
    out_arrs = sharded(*concat_in, *zeros_dev)
    out_arrs = [np.asarray(o) for o in out_arrs]
    return [
        {name: out_arrs[i].reshape(8, *out_avals[i].shape)[c]
         for i, name in enumerate(out_names)}
        for c in range(8)
    ]


def build_in_maps(A, B, W1, b1, W2, b2):
    A = np.ascontiguousarray(np.asarray(A, dtype=np.float32))
    B = np.ascontiguousarray(np.asarray(B, dtype=np.float32))
    W1 = np.asarray(W1, dtype=np.float32)
    b1 = np.asarray(b1, dtype=np.float32)
    W2 = np.asarray(W2, dtype=np.float32)
    b2 = np.asarray(b2, dtype=np.float32)
    nbatch, seq, d = A.shape
    assert (nbatch, seq, d) == (4, N, D), (nbatch, seq, d)

    w1r = np.ascontiguousarray(W1)                                # [128, 256]
    w2r = np.ascontiguousarray(
        W2.reshape(2, 128, 2, 128).transpose(1, 0, 2, 3).reshape(128, 512))
    b1c = np.ascontiguousarray(b1.reshape(2, 128).T)              # [128, 2]
    b2c = np.ascontiguousarray(b2.reshape(2, 128).T)

    in_maps = []
    for core in range(8):
        b_i, half = divmod(core, 2)
        Ah = A[b_i, half * M:(half + 1) * M]                      # [2048, 128]
        Bf = B[b_i]                                               # [4096, 128]
        in_maps.append({
            "atb": np.ascontiguousarray(Ah.T),
            "btb": np.ascontiguousarray(Bf.T),
            "anr": np.ascontiguousarray(
                Ah.reshape(MC, 128, 128).transpose(1, 0, 2).reshape(128, M)),
            "bnr": np.ascontiguousarray(
                Bf.reshape(NB, 128, 128).transpose(1, 0, 2).reshape(128, N)),
            "w1": w1r, "w2": w2r, "b1": b1c, "b2": b2c,
        })
    return in_maps


def kernel(A, B, W1, b1, W2, b2):
    in_maps = build_in_maps(A, B, W1, b1, W2, b2)
    results = run_cores(in_maps)

    beta = np.empty((4, N, D), dtype=np.float32)
    alpha = np.empty((4, N, D), dtype=np.float32)
    for b_i in range(4):
        r0 = results[2 * b_i]
        r1 = results[2 * b_i + 1]
        beta[b_i, :M] = r0["beta"]
        beta[b_i, M:] = r1["beta"]
        num = r0["alphat"] + r1["alphat"]                          # [128, 4096]
        colp = r0["cols"] + r1["cols"]                             # [128, 64]
        csum = colp.reshape(128, NB, 2).sum(axis=2)                # [128, 32]
        # csum[p, j] corresponds to n = j*128 + p
        alpha[b_i] = (num / csum.T.reshape(1, N)).T.reshape(N, D)
    return beta, alpha


if __name__ == "__main__":
    rng = np.random.default_rng(0)
    A = rng.standard_normal((4, N, D)).astype(np.float32)
    B = rng.standard_normal((4, N, D)).astype(np.float32)
    s1, s2 = 1.0 / np.sqrt(D), 1.0 / np.sqrt(H)
    W1 = rng.uniform(-s1, s1, (D, H)).astype(np.float32)
    b1 = rng.uniform(-s1, s1, H).astype(np.float32)
    W2 = rng.uniform(-s2, s2, (H, H)).astype(np.float32)
    b2 = rng.uniform(-s2, s2, H).astype(np.float32)
    beta, alpha = kernel(A=A, B=B, W1=W1, b1=b1, W2=W2, b2=b2)
    print("beta", beta.shape, "alpha", alpha.shape)


# revision 20
# speedup vs baseline: 10924.4196x; 123.2677x over previous
"""Trainium2 Bass kernel for nn_Attend (decomposable attention).

Computation (reference):
    f_A = relu(relu(A @ W1 + b1) @ W2 + b2)      [b, m, h]
    f_B = relu(relu(B @ W1 + b1) @ W2 + b2)      [b, n, h]
    e = f_A @ f_B^T                               [b, m, n]
    beta  = softmax(e, axis=-1) @ B               [b, m, d]
    alpha = softmax(e, axis=-2)^T @ A             [b, n, d]
    returns (beta, alpha)

Shapes: b=4, m=n=4096, d=128, h=256. Scores e lie in ~[0.5, 8.3] so
exp() needs no max-subtraction: softmax = exp(e)/sum(exp(e)) directly,
and cross-shard softmax combines are plain sums of partials.

Sharding: 8 cores = (batch, m-half). Each core handles 2048 m-rows of
one batch against all 4096 n. beta is fully local (row softmax over n).
alpha needs a sum over m across the 2 cores of a batch: each core emits
unnormalized alpha^T partials + column-sum partials; the host adds the
two partials and divides (exact).

On-core dataflow (all heavy matmuls in float32r: full PE rate at
N=512 moving dim, ~1.6e-4 relative error):
  1. MLP in transposed layout: fB^T [256, 4096], fA^T [256, 2048] as
     2x128-partition f32r tiles. Bias+relu fused on DVE (tensor_scalar
     add+max from PSUM).
  2. alpha phase (natural layout, m-chunk outer, n split in 2 halves):
     E chunk [128m, 512n] on PE -> ACT exp -> P natural (f32r) with
     fused per-m rowsum (accum_out); alpha^T [128d, 2048n] accumulates
     in PSUM over the 16 m-chunks: lhsT=A-natural-block, rhs=P.
  3. beta phase (transposed layout, n-block outer): E^T [128n, 512m]
     on PE -> ACT exp -> P^T (f32r) with fused per-n colsum partials;
     beta^T [128d, 2048m] accumulates in PSUM over the 32 n-blocks:
     lhsT=B-natural-block, rhs=P^T.
  4. finalize: beta^T -> PE transpose per 128-block -> ACT copy scaled
     by 1/rowsum (per-partition scale) -> natural [2048, 128] output.
"""

import sys

import numpy as np

if "/opt/trn_rl_repo" not in sys.path:
    sys.path.insert(0, "/opt/trn_rl_repo")

import concourse.bass as bass  # noqa: E402
import concourse.mybir as mybir  # noqa: E402
import concourse.tile as tile  # noqa: E402
from concourse import bacc  # noqa: E402
from concourse.masks import make_identity  # noqa: E402

F32 = mybir.dt.float32
F32R = mybir.dt.float32r
EXP = mybir.ActivationFunctionType.Exp
COPY = mybir.ActivationFunctionType.Copy
RELU = mybir.ActivationFunctionType.Relu

D = 128      # model dim
H = 256      # hidden dim
M = 2048     # rows per core (half of 4096)
N = 4096     # full sequence
MC = M // 128   # m chunks per core (16)
NB = N // 128   # n blocks (32)

_CACHE = {}


def _mlp_transposed(nc, pools, xT, fT0, fT1, w1, w2, b1c, b2c, zero, seq):
    """fT = relu(W2^T @ relu(W1^T @ xT + b1) + b2) in transposed layout.

    xT: [128 d, seq] f32r;  fT0/fT1: [128, seq] f32r (h split in 2 blocks).
    h1 bias+relu runs on ACT, h2 bias+relu on DVE (splits the elementwise
    load so neither engine gates the PE).
    """
    mlp_ps, h1_pool = pools
    h1 = [h1_pool.tile([128, seq], F32R, tag=f"h1_{i}", name=f"h1_{i}")
          for i in range(2)]
    nchunks = seq // 512
    for s in range(nchunks):
        sl = bass.ts(s, 512)
        for i in range(2):
            ps = mlp_ps.tile([128, 512], F32, tag="mlp")
            nc.tensor.matmul(ps, w1[:, bass.ts(i, 128)], xT[:, sl],
                             start=True, stop=True)
            nc.scalar.activation(h1[i][:, sl], ps, RELU,
                                 bias=b1c[:, i:i + 1])
        for i, fT in enumerate((fT0, fT1)):
            if s == 0:
                continue
            ps = mlp_ps.tile([128, 512], F32, tag="mlp")
            sl_prev = bass.ts(s - 1, 512)
            for kh in range(2):
                nc.tensor.matmul(ps, w2[:, bass.ts(kh * 2 + i, 128)],
                                 h1[kh][:, sl_prev],
                                 start=(kh == 0), stop=(kh == 1))
            nc.vector.tensor_scalar(
                out=fT[:, sl_prev], in0=ps,
                scalar1=b2c[:, i:i + 1], scalar2=zero,
                op0=mybir.AluOpType.add, op1=mybir.AluOpType.max)
    sl = bass.ts(nchunks - 1, 512)
    for i, fT in enumerate((fT0, fT1)):
        ps = mlp_ps.tile([128, 512], F32, tag="mlp")
        for kh in range(2):
            nc.tensor.matmul(ps, w2[:, bass.ts(kh * 2 + i, 128)],
                             h1[kh][:, sl],
                             start=(kh == 0), stop=(kh == 1))
        nc.vector.tensor_scalar(
            out=fT[:, sl], in0=ps,
            scalar1=b2c[:, i:i + 1], scalar2=zero,
            op0=mybir.AluOpType.add, op1=mybir.AluOpType.max)


def _build():
    """Build + compile the per-core Bass program (same NEFF on all 8 cores)."""
    nc = bacc.Bacc(None, target_bir_lowering=False)

    # inputs (f32r tensors take plain fp32 host arrays)
    atb = nc.declare_dram_parameter("atb", [128, M], F32R, isOutput=False)
    btb = nc.declare_dram_parameter("btb", [128, N], F32R, isOutput=False)
    anr = nc.declare_dram_parameter("anr", [128, M], F32R, isOutput=False)
    bnr = nc.declare_dram_parameter("bnr", [128, N], F32R, isOutput=False)
    w1 = nc.declare_dram_parameter("w1", [128, H], F32R, isOutput=False)
    w2 = nc.declare_dram_parameter("w2", [128, 2 * H], F32R, isOutput=False)
    b1 = nc.declare_dram_parameter("b1", [128, 2], F32, isOutput=False)
    b2 = nc.declare_dram_parameter("b2", [128, 2], F32, isOutput=False)
    # outputs
    beta_d = nc.declare_dram_parameter("beta", [M, D], F32, isOutput=True)
    alphat_d = nc.declare_dram_parameter("alphat", [128, N], F32, isOutput=True)
    cols_d = nc.declare_dram_parameter("cols", [128, 2 * NB], F32, isOutput=True)

    with tile.TileContext(nc) as tc, \
         tc.tile_pool(name="const", bufs=1) as const:
        # persistent SBUF tensors
        atb_sb = const.tile([128, M], F32R, tag="atb")
        btb_sb = const.tile([128, N], F32R, tag="btb")
        anr_sb = const.tile([128, M], F32R, tag="anr")
        bnr_sb = const.tile([128, N], F32R, tag="bnr")
        w1_sb = const.tile([128, H], F32R, tag="w1")
        w2_sb = const.tile([128, 2 * H], F32R, tag="w2")
        b1_sb = const.tile([128, 2], F32, tag="b1")
        b2_sb = const.tile([128, 2], F32, tag="b2")

        zero = const.tile([128, 1], F32, tag="zero")
        nc.vector.memset(zero, 0.0)
        # trigger the exp table-set load on ACT immediately (overlaps with
        # the input DMAs instead of stalling the first real exp)
        dummy = const.tile([128, 1], F32, tag="dummy")
        nc.scalar.activation(dummy, zero, EXP)

        # DMA issue costs ~650ns/instr on the SP sequencer and transfers
        # drain through a mostly-serial DMA pipe, so: small weight tensors
        # and the first MLP operand chunk first, big tensors in few chunks,
        # ordered by first use.
        nc.sync.dma_start(w1_sb, w1[:])
        nc.sync.dma_start(btb_sb[:, 0:512], btb[:, 0:512])
        nc.sync.dma_start(b1_sb, b1[:])
        nc.sync.dma_start(w2_sb, w2[:])
        nc.sync.dma_start(b2_sb, b2[:])
        nc.sync.dma_start(btb_sb[:, 512:1024], btb[:, 512:1024])
        nc.sync.dma_start(btb_sb[:, 1024:2048], btb[:, 1024:2048])
        nc.sync.dma_start(btb_sb[:, 2048:4096], btb[:, 2048:4096])
        nc.sync.dma_start(atb_sb, atb[:])
        nc.sync.dma_start(anr_sb, anr[:])
        nc.sync.dma_start(bnr_sb, bnr[:])

        fbt = [const.tile([128, N], F32R, tag=f"fbt{k}", name=f"fbt{k}")
               for k in range(2)]
        fat = [const.tile([128, M], F32R, tag=f"fat{k}", name=f"fat{k}")
               for k in range(2)]
        rows_sb = const.tile([128, 4 * MC], F32, tag="rows")
        cols_sb = const.tile([128, 2 * NB], F32, tag="cols")

        # ---- phase 1: MLPs ----
        with tc.tile_pool(name="mlp_ps", bufs=4, space="PSUM") as mlp_ps, \
             tc.tile_pool(name="h1", bufs=1) as h1_pool:
            pools = (mlp_ps, h1_pool)
            _mlp_transposed(nc, pools, btb_sb, fbt[0], fbt[1],
                            w1_sb, w2_sb, b1_sb, b2_sb, zero, N)
            _mlp_transposed(nc, pools, atb_sb, fat[0], fat[1],
                            w1_sb, w2_sb, b1_sb, b2_sb, zero, M)

        # ---- phases 2+3 share one PSUM/SBUF pool set: the beta phase's
        # tiles reuse the alpha phase's slots as they free, with no
        # pool-close barrier at the transition ----
        rs_tmp = const.tile([128, 2 * MC], F32, tag="rst")
        rs = const.tile([128, MC], F32, tag="rs")
        inv_rs = const.tile([128, MC], F32, tag="irs")
        ident = const.tile([128, 128], F32, tag="id")
        make_identity(nc, ident)
        with tc.tile_pool(name="acc_ps", bufs=1, space="PSUM") as acc_pool, \
             tc.tile_pool(name="et_ps", bufs=2, space="PSUM") as et_pool, \
             tc.tile_pool(name="pp", bufs=4) as ppool, \
             tc.tile_pool(name="stage", bufs=4) as stage, \
             tc.tile_pool(name="fin", bufs=1) as fin:
            # -- alpha + rowsums --
            for nh in range(2):
                # 4 single-bank accumulators so finished slices copy out and
                # free PSUM without waiting for the whole [128, 2048] strip
                alpha_ps = [acc_pool.tile([128, 512], F32, tag=f"acc{q}",
                                          name=f"acc{q}") for q in range(4)]
                for c in range(MC):
                    p_c = ppool.tile([128, 2048], F32R, tag="P")
                    for sub in range(2):
                        et = et_pool.tile([128, 1024], F32, tag="et")
                        for h2 in range(2):
                            ncol = nh * 2048 + sub * 1024 + h2 * 512
                            for k in range(2):
                                nc.tensor.matmul(
                                    et[:, bass.ts(h2, 512)],
                                    fat[k][:, bass.ts(c, 128)],
                                    fbt[k][:, ncol:ncol + 512],
                                    start=(k == 0), stop=(k == 1))
                        nc.scalar.activation(
                            p_c[:, bass.ts(sub, 1024)], et, EXP,
                            accum_out=rows_sb[:, nh * 2 * MC + c * 2 + sub
                                              : nh * 2 * MC + c * 2 + sub + 1])
                    for q in range(4):
                        nc.tensor.matmul(
                            alpha_ps[q],
                            anr_sb[:, bass.ts(c, 128)],
                            p_c[:, bass.ts(q, 512)],
                            start=(c == 0), stop=(c == MC - 1))
                for q in range(4):
                    a_sb = stage.tile([128, 512], F32, tag="as")
                    nc.vector.tensor_copy(a_sb, alpha_ps[q])
                    nc.sync.dma_start(
                        alphat_d[:, nh * 2048 + q * 512:nh * 2048 + (q + 1) * 512],
                        a_sb)

            # rowsum -> 1/rowsum as soon as the alpha-phase exps finish
            # (off the critical path of the beta phase)
            nc.vector.tensor_add(rs_tmp, rows_sb[:, 0:2 * MC],
                                 rows_sb[:, 2 * MC:4 * MC])
            rs_pairs = rs_tmp.rearrange("p (c s) -> p c s", s=2)
            nc.vector.tensor_add(rs, rs_pairs[:, :, 0], rs_pairs[:, :, 1])
            nc.vector.reciprocal(inv_rs, rs)

            # -- beta + colsums (accumulators reuse the acc0-3 slots) --
            beta_ps = [acc_pool.tile([128, 512], F32, tag=f"acc{q}",
                                     name=f"bacc{q}") for q in range(4)]
            betat_sb = fin.tile([128, M], F32, tag="bt")
            for j in range(NB):
                pt_j = ppool.tile([128, M], F32R, tag="P")
                for sub in range(2):
                    et = et_pool.tile([128, 1024], F32, tag="et")
                    for h2 in range(2):
                        mcol = sub * 1024 + h2 * 512
                        for k in range(2):
                            nc.tensor.matmul(
                                et[:, bass.ts(h2, 512)],
                                fbt[k][:, bass.ts(j, 128)],
                                fat[k][:, mcol:mcol + 512],
                                start=(k == 0), stop=(k == 1))
                    nc.scalar.activation(
                        pt_j[:, bass.ts(sub, 1024)], et, EXP,
                        accum_out=cols_sb[:, j * 2 + sub:j * 2 + sub + 1])
                for q in range(4):
                    nc.tensor.matmul(
                        beta_ps[q],
                        bnr_sb[:, bass.ts(j, 128)],
                        pt_j[:, bass.ts(q, 512)],
                        start=(j == 0), stop=(j == NB - 1))
            nc.sync.dma_start(cols_d[:], cols_sb)
            for q in range(4):
                nc.vector.tensor_copy(betat_sb[:, bass.ts(q, 512)],
                                      beta_ps[q])

            # -- finalize beta: transpose blocks + scale by 1/rowsum;
            # per-block DMA so stores overlap the transposes --
            beta_out = fin.tile([128, MC, 128], F32, tag="bout")
            beta_nat = beta_d[:].rearrange("(c p) d -> p c d", p=128)
            for c in range(MC):
                tr = et_pool.tile([128, 128], F32, tag="et", name=f"tr{c}")
                nc.tensor.transpose(tr, betat_sb[:, bass.ts(c, 128)], ident)
                nc.vector.tensor_scalar_mul(beta_out[:, c, :], tr,
                                            inv_rs[:, c:c + 1])
                nc.sync.dma_start(beta_nat[:, c, :], beta_out[:, c, :])

    nc.compile()
    return nc


def _get_nc():
    if "nc" not in _CACHE:
        _CACHE["nc"] = _build()
    return _CACHE["nc"]


def _get_runner():
    """Jitted 8-core shard_map executor built once (mirrors
    bass2jax.run_bass_via_pjrt, but cacheable across calls)."""
    if "runner" in _CACHE:
        return _CACHE["runner"]
    import jax
    from jax.sharding import Mesh, PartitionSpec
    from jax.experimental.shard_map import shard_map
    import concourse.mybir as mb
    from concourse.bass2jax import (
        _bass_exec_p, install_neuronx_cc_hook, partition_id_tensor)

    nc = _get_nc()
    install_neuronx_cc_hook()

    in_names, out_names, out_avals = [], [], []
    partition_name = (nc.partition_id_tensor.name
                      if nc.partition_id_tensor else None)
    for alloc in nc.m.functions[0].allocations:
        if not isinstance(alloc, mb.MemoryLocationSet):
            continue
        name = alloc.memorylocations[0].name
        if alloc.kind == "ExternalInput":
            if name != partition_name:
                in_names.append(name)
        elif alloc.kind == "ExternalOutput":
            out_names.append(name)
            out_avals.append(jax.core.ShapedArray(
                tuple(alloc.tensor_shape), mb.dt.np(alloc.dtype)))
    n_params = len(in_names)
    zero_outs = [np.zeros((8 * a.shape[0], *a.shape[1:]), a.dtype)
                 for a in out_avals]
    all_in_names = in_names + out_names
    if partition_name is not None:
        all_in_names = all_in_names + [partition_name]

    def _body(*args):
        operands = list(args)
        if partition_name is not None:
            operands.append(partition_id_tensor())
        return tuple(_bass_exec_p.bind(
            *operands,
            out_avals=tuple(out_avals),
            in_names=tuple(all_in_names),
            out_names=tuple(out_names),
            lowering_input_output_aliases=(),
            sim_require_finite=True,
            sim_require_nnan=True,
            nc=nc,
        ))

    devices = jax.devices()[:8]
    mesh = Mesh(np.asarray(devices), ("core",))
    nin = n_params + len(out_names)
    sharded = jax.jit(shard_map(
        _body, mesh=mesh,
        in_specs=(PartitionSpec("core"),) * nin,
        out_specs=(PartitionSpec("core"),) * len(out_names),
        check_rep=False))
    zeros_dev = [jax.device_put(z) for z in zero_outs]
    _CACHE["runner"] = (sharded, in_names, out_names, out_avals, zeros_dev)
    return _CACHE["runner"]


def run_cores(in_maps):
    """Run the 8-core program; returns list of per-core output dicts."""
    import jax
    sharded, in_names, out_names, out_avals, zeros_dev = _get_runner()
    concat_in = [np.concatenate([m[name] for m in in_maps], axis=0)
                 for name in in_names]
    out_arrs = sharded(*concat_in, *zeros_dev)
    out_arrs = [np.asarray(o) for o in out_arrs]
    return [
        {name: out_arrs[i].reshape(8, *out_avals[i].shape)[c]
         for i, name in enumerate(out_names)}
        for c in range(8)
    ]


def build_in_maps(A, B, W1, b1, W2, b2):
    A = np.ascontiguousarray(np.asarray(A, dtype=np.float32))
    B = np.ascontiguousarray(np.asarray(B, dtype=np.float32))
    W1 = np.asarray(W1, dtype=np.float32)
    b1 = np.asarray(b1, dtype=np.float32)
    W2 = np.asarray(W2, dtype=np.float32)
    b2 = np.asarray(b2, dtype=np.float32)
    nbatch, seq, d = A.shape
    assert (nbatch, seq, d) == (4, N, D), (nbatch, seq, d)

    w1r = np.ascontiguousarray(W1)                                # [128, 256]
    w2r = np.ascontiguousarray(
        W2.reshape(2, 128, 2, 128).transpose(1, 0, 2, 3).reshape(128, 512))
    b1c = np.ascontiguousarray(b1.reshape(2, 128).T)              # [128, 2]
    b2c = np.ascontiguousarray(b2.reshape(2, 128).T)

    in_maps = []
    for core in range(8):
        b_i, half = divmod(core, 2)
        Ah = A[b_i, half * M:(half + 1) * M]                      # [2048, 128]
        Bf = B[b_i]                                               # [4096, 128]
        in_maps.append({
            "atb": np.ascontiguousarray(Ah.T),
            "btb": np.ascontiguousarray(Bf.T),
            "anr": np.ascontiguousarray(
                Ah.reshape(MC, 128, 128).transpose(1, 0, 2).reshape(128, M)),
            "bnr": np.ascontiguousarray(
                Bf.reshape(NB, 128, 128).transpose(1, 0, 2).reshape(128, N)),
            "w1": w1r, "w2": w2r, "b1": b1c, "b2": b2c,
        })
    return in_maps


def kernel(A, B, W1, b1, W2, b2):
    in_maps = build_in_maps(A, B, W1, b1, W2, b2)
    results = run_cores(in_maps)

    beta = np.empty((4, N, D), dtype=np.float32)
    alpha = np.empty((4, N, D), dtype=np.float32)
    for b_i in range(4):
        r0 = results[2 * b_i]
        r1 = results[2 * b_i + 1]
        beta[b_i, :M] = r0["beta"]
        beta[b_i, M:] = r1["beta"]
        num = r0["alphat"] + r1["alphat"]                          # [128, 4096]
        colp = r0["cols"] + r1["cols"]                             # [128, 64]
        csum = colp.reshape(128, NB, 2).sum(axis=2)                # [128, 32]
        # csum[p, j] corresponds to n = j*128 + p
        alpha[b_i] = (num / csum.T.reshape(1, N)).T.reshape(N, D)
    return beta, alpha


if __name__ == "__main__":
    rng = np.random.default_rng(0)
    A = rng.standard_normal((4, N, D)).astype(np.float32)
    B = rng.standard_normal((4, N, D)).astype(np.float32)
    s1, s2 = 1.0 / np.sqrt(D), 1.0 / np.sqrt(H)
    W1 = rng.uniform(-s1, s1, (D, H)).astype(np.float32)
    b1 = rng.uniform(-s1, s1, H).astype(np.float32)
    W2 = rng.uniform(-s2, s2, (H, H)).astype(np.float32)
    b2 = rng.uniform(-s2, s2, H).astype(np.float32)
    beta, alpha = kernel(A=A, B=B, W1=W1, b1=b1, W2=W2, b2=b2)
    print("beta", beta.shape, "alpha", alpha.shape)
